# revision 22
# baseline (speedup 1.0000x reference)
"""Multi-head attention kernel for Trainium2 (8 NeuronCores).

Problem: B=2, S=2048, 16 heads, d_head=64, shared 64x64 per-head projections.
  out = softmax((q Wq^T)(k Wk^T)^T / 8) @ (v Wv^T), per (batch, head).

Sharding: 32 (b,h) pairs -> 4 pairs per core (data + head parallel).

Host folding: scores = qh (Wq^T Wk / 8) kh^T, so M1 := Wq^T Wk / 8 is folded
into q on the host (qtT = M1^T qhT) and Wv is folded into v (vt = vh Wv^T).
q/k/v ship as fp16 (the 2e-2 tolerance dwarfs fp16 noise); outputs return as
unnormalized numerator + row-sum denominator (ones column in vt) and the
final divide happens on the host.

Device, per (pair, 512-wide sq chunk):
  - scores S^T tile [sk=128, sq=512] per sk-tile via fp16 matmul (full-rate).
  - exp in 8 groups of 2 sk-tiles; even groups on ACT (table exp -> fp16),
    odd groups on DVE via the Schraudolph bit trick: fp16_bits(e^x) ~=
    uint16(round(1477.32*x + 15300.5)), one tensor_scalar op (uint16 so
    deeply-negative scores saturate to +0.0 instead of NaN bit patterns).  The two engines
    run concurrently so exp is not the bottleneck; the DVE share carries a
    ~1.8% sawtooth error (measured total ~1.3e-2 rel L2, well under 2e-2).
  - attention @ vt streams N=65 (64 + ones column) fp16 rows per (sk-tile,
    sq-block) into a single PSUM bank: the chunk's first accumulation runs
    with start=True, which zeroes the whole 2KB bank (verified on HW), so
    the four interleaved sq-block accumulation groups share the bank with
    start=False and no zero-region conflicts.
  - O work for chunk t is spread across chunks t+1/t+2 per O_SCHED (sq-block
    3 deferred a full chunk) which removes the chunk-boundary semaphore
    stalls; ACT then copies PSUM->SBUF and the result DMAs out.
"""

import numpy as np

import concourse.bacc as bacc
import concourse.mybir as mybir
import concourse.tile as tile
from concourse.bass_utils import run_bass_kernel_spmd

F32 = mybir.dt.float32
F16 = mybir.dt.float16
U16 = mybir.dt.uint16

N_CORES = 8
B, S, D_EMBED = 2, 2048, 1024
N_HEADS = 16
D = 64  # d_head
NPAIR = (B * N_HEADS) // N_CORES  # 4 (b,h) pairs per core
NT = S // 128  # 16 sk-tiles
NCHUNK = S // 512  # 4 sq chunks
NG = 8  # exp groups per chunk, 2 sk-tiles each
DVE_GROUPS = (1, 3, 5, 7)  # these groups take the DVE fast-exp path
N_WARM = 4  # PE clock-gate warmup matmuls
# O-block schedule: (chunk_offset, sqb, g_slot) — at chunk t, slot g, emit
# the O block for chunk t-chunk_offset / sq-block sqb. The epilogue for a
# chunk is emitted right after its sqb=3 entry.
O_SCHED = ((1, 0, 2), (1, 1, 4), (1, 2, 6), (2, 3, 0))

# fp16 Schraudolph: bits(e^x) ~= round(2^10*log2(e)*x + 15*2^10 + sigma),
# sigma tuned to center the piecewise-linear sawtooth (RMS ~1.8%).
EXP_A = 1024.0 * float(np.log2(np.e))
EXP_B = 15.0 * 1024.0 - 59.5

_NC_CACHE = {}


def build_nc():
    nc = bacc.Bacc("TRN2", target_bir_lowering=False)
    # k and q ship interleaved in one tensor so each pair costs one DMA
    kq_d = nc.dram_tensor("kq", [NPAIR, 2, D, S], F16, kind="ExternalInput").ap()
    vs_d = nc.dram_tensor("vs", [NPAIR, S, D + 1], F16, kind="ExternalInput").ap()
    out_d = nc.dram_tensor("out", [NPAIR, S, D + 1], F32, kind="ExternalOutput").ap()

    with tile.TileContext(nc) as tc:
        with (
            tc.tile_pool(name="const", bufs=1) as const,
            tc.tile_pool(name="io", bufs=2) as io,
            tc.tile_pool(name="pt", bufs=3 * NG) as pt_pool,
            tc.tile_pool(name="ob", bufs=3) as ob_pool,
            tc.tile_pool(name="st_ps", bufs=3, space="PSUM") as st_ps,
            tc.tile_pool(name="o_ps", bufs=2, space="PSUM") as o_ps_pool,
        ):
            z_sb = const.tile([128, 512], F16)
            nc.vector.memset(z_sb, 0.0)

            pairs = {}

            def alloc_pair(p):
                kq = io.tile([D, 2, S], F16, tag="kq", name=f"kq{p}")
                pairs[p] = {
                    "kq": kq,
                    "khT": kq[:, 0, :],
                    "qtT": kq[:, 1, :],
                    "v": io.tile([128, NT, D + 1], F16, tag="v", name=f"v{p}"),
                }

            def load_q(p):
                nc.sync.dma_start(
                    out=pairs[p]["kq"],
                    in_=kq_d[p].rearrange("t d s -> d t s"),
                )

            def load_v(p):
                nc.sync.dma_start(
                    out=pairs[p]["v"],
                    in_=vs_d[p].rearrange("(t r) d -> r t d", r=128),
                )

            # per-chunk pipeline state: t -> {pt tiles, o accum, pair tiles}
            cstate = {}

            def emit_scores_group(p, c, g, t):
                stt = st_ps.tile([128, 1024], F32, tag="st", name=f"st{t}_{g}")
                st_p = pairs[p]
                for j in range(2):
                    sk = 2 * g + j
                    nc.tensor.matmul(
                        stt[:, j * 512 : (j + 1) * 512],
                        st_p["khT"][:, sk * 128 : (sk + 1) * 128],
                        st_p["qtT"][:, c * 512 : (c + 1) * 512],
                        start=True,
                        stop=True,
                    )
                ptile = pt_pool.tile([128, 1024], F16, tag="pt", name=f"pt{t}_{g}")
                if g in DVE_GROUPS:
                    nc.vector.tensor_scalar(
                        out=ptile.bitcast(U16),
                        in0=stt,
                        scalar1=EXP_A,
                        scalar2=EXP_B,
                        op0=mybir.AluOpType.mult,
                        op1=mybir.AluOpType.add,
                    )
                else:
                    nc.scalar.activation(
                        ptile, stt, mybir.ActivationFunctionType.Exp
                    )
                cstate[t]["pt"].append(ptile)

            def emit_o_block(t, sqb, sk_lo=0, sk_hi=NT):
                cs = cstate[t]
                if "o" not in cs:
                    # first write below carries start=True, which zeroes the
                    # whole 2KB PSUM bank (verified on HW), so the other
                    # sq-blocks accumulate onto zeros with start=False
                    cs["o"] = o_ps_pool.tile(
                        [128, 4, D + 1], F32, tag="o", name=f"o{t}"
                    )
                    cs["zeroed"] = False
                o, v = cs["o"], cs["v"]
                for sk in range(sk_lo, sk_hi):
                    ptile = cs["pt"][sk // 2]
                    col = (sk % 2) * 512 + sqb * 128
                    nc.tensor.matmul(
                        o[:, sqb, :],
                        ptile[:, col : col + 128],
                        v[:, sk, :],
                        start=not cs["zeroed"],
                        stop=(sk == NT - 1),
                        skip_group_check=True,
                    )
                    cs["zeroed"] = True

            def emit_epilogue(t):
                cs = cstate.pop(t)
                ob = ob_pool.tile([128, 4, D + 1], F32, tag="ob", name=f"ob{t}")
                nc.scalar.copy(ob, cs["o"])
                p, c = cs["p"], cs["c"]
                nc.sync.dma_start(
                    out=out_d[p].rearrange("(t r) d -> r t d", r=128)[
                        :, c * 4 : (c + 1) * 4, :
                    ],
                    in_=ob,
                )

            # ---- prologue: pair 0 loads + PE clock warmup ----
            # first quarter of khT/qtT ships first so chunk 0 can start while
            # the rest streams in behind it
            alloc_pair(0)
            nc.sync.dma_start(
                out=pairs[0]["kq"][:, :, 0:512],
                in_=kq_d[0].rearrange("t d s -> d t s")[:, :, 0:512],
            )
            nc.sync.dma_start(
                out=pairs[0]["kq"][:, :, 512:2048],
                in_=kq_d[0].rearrange("t d s -> d t s")[:, :, 512:2048],
            )
            load_v(0)
            for w in range(N_WARM):
                wt = st_ps.tile([128, 1024], F32, tag="st", name=f"warm{w}")
                nc.tensor.matmul(
                    wt[:, 0:512], z_sb[:, 0:128], z_sb, start=True, stop=True
                )

            # ---- main pipeline ----
            for p in range(NPAIR):
                for c in range(NCHUNK):
                    t = p * NCHUNK + c
                    cstate[t] = {"pt": [], "p": p, "c": c, "v": pairs[p]["v"]}
                    if p + 1 < NPAIR:
                        if c == 0:
                            alloc_pair(p + 1)
                            load_q(p + 1)
                        elif c == 1:
                            load_v(p + 1)
                    for g in range(NG):
                        emit_scores_group(p, c, g, t)
                        for off, sqb, gs in O_SCHED:
                            if gs == g and t - off >= 0:
                                emit_o_block(t - off, sqb)
                                if sqb == 3:
                                    emit_epilogue(t - off)
            # ---- tail: flush O blocks still owed by the schedule, newest
            # chunk in exp-trailing quarters ----
            t_last = NPAIR * NCHUNK - 1
            max_off = max(off for off, _, _ in O_SCHED)
            for t_o in range(t_last - max_off + 1, t_last):
                for off, sqb, gs in O_SCHED:
                    if t_o + off > t_last:
                        emit_o_block(t_o, sqb)
                        if sqb == 3:
                            emit_epilogue(t_o)
            for quarter in range(4):
                for sqb in range(4):
                    emit_o_block(
                        t_last, sqb, sk_lo=quarter * 4, sk_hi=(quarter + 1) * 4
                    )
            emit_epilogue(t_last)
    nc.finalize()
    return nc


def _host_prep(k, q, v, Wk, Wq, Wv):
    m1 = ((Wq.T @ Wk) / np.sqrt(np.float32(D))).astype(np.float32)

    def split_heads_T(x):
        return (
            x.reshape(B, S, N_HEADS, D)
            .transpose(0, 2, 3, 1)
            .reshape(B * N_HEADS, D, S)
        )

    khT = split_heads_T(k).astype(np.float16)
    qtT = (m1.T @ split_heads_T(q)).astype(np.float16)
    kq = np.ascontiguousarray(np.stack([khT, qtT], axis=1))  # [BH, 2, D, S]
    vh = v.reshape(B, S, N_HEADS, D).transpose(0, 2, 1, 3).reshape(B * N_HEADS, S, D)
    vt = np.empty((B * N_HEADS, S, D + 1), dtype=np.float16)
    vt[:, :, :D] = vh @ Wv.T
    vt[:, :, D] = 1.0
    return kq, vt


def kernel(k, q, v, Wk, Wq, Wv):
    k = np.asarray(k, dtype=np.float32)
    q = np.asarray(q, dtype=np.float32)
    v = np.asarray(v, dtype=np.float32)
    Wk = np.asarray(Wk, dtype=np.float32)
    Wq = np.asarray(Wq, dtype=np.float32)
    Wv = np.asarray(Wv, dtype=np.float32)

    kq, vt = _host_prep(k, q, v, Wk, Wq, Wv)

    if "nc" not in _NC_CACHE:
        _NC_CACHE["nc"] = build_nc()
    nc = _NC_CACHE["nc"]

    in_maps = []
    for core in range(N_CORES):
        sl = slice(core * NPAIR, (core + 1) * NPAIR)
        in_maps.append({"kq": kq[sl], "vs": vt[sl]})

    res = run_bass_kernel_spmd(nc, in_maps, core_ids=list(range(N_CORES)))
    outs = np.stack([r["out"] for r in res.results])  # [8, NPAIR, S, D+1]
    outs = outs.reshape(B * N_HEADS, S, D + 1)
    o = outs[:, :, :D] / outs[:, :, D:]
    out = (
        o.reshape(B, N_HEADS, S, D).transpose(0, 2, 1, 3).reshape(B, S, D_EMBED)
    )
    return out.astype(np.float32)


# revision 26
# speedup vs baseline: 1.0098x; 1.0098x over previous
"""Multi-head attention kernel for Trainium2 (8 NeuronCores).

Problem: B=2, S=2048, 16 heads, d_head=64, shared 64x64 per-head projections.
  out = softmax((q Wq^T)(k Wk^T)^T / 8) @ (v Wv^T), per (batch, head).

Sharding: 32 (b,h) pairs -> 4 pairs per core (data + head parallel).

Host folding: scores = qh (Wq^T Wk / 8) kh^T, so M1 := Wq^T Wk / 8 is folded
into q on the host (qtT = M1^T qhT) and Wv is folded into v (vt = vh Wv^T).
q/k/v ship as fp16 (the 2e-2 tolerance dwarfs fp16 noise); outputs return as
unnormalized numerator + row-sum denominator (ones column in vt) and the
final divide happens on the host.

Device, per (pair, 512-wide sq chunk):
  - scores S^T tile [sk=128, sq=512] per sk-tile via fp16 matmul (full-rate).
  - exp in 8 groups of 2 sk-tiles; even groups on ACT (table exp -> fp16),
    odd groups on DVE via the Schraudolph bit trick: fp16_bits(e^x) ~=
    uint16(round(1477.32*x + 15300.5)), one tensor_scalar op (uint16 so
    deeply-negative scores saturate to +0.0 instead of NaN bit patterns).  The two engines
    run concurrently so exp is not the bottleneck; the DVE share carries a
    ~1.8% sawtooth error (measured total ~1.3e-2 rel L2, well under 2e-2).
  - attention @ vt streams N=65 (64 + ones column) fp16 rows per (sk-tile,
    sq-block) into a single PSUM bank: the chunk's first accumulation runs
    with start=True, which zeroes the whole 2KB bank (verified on HW), so
    the four interleaved sq-block accumulation groups share the bank with
    start=False and no zero-region conflicts.
  - O work for chunk t is spread across chunks t+1/t+2 per O_SCHED (sq-block
    3 deferred a full chunk) which removes the chunk-boundary semaphore
    stalls; ACT then copies PSUM->SBUF and the result DMAs out.
"""

import numpy as np

import concourse.bacc as bacc
import concourse.mybir as mybir
import concourse.tile as tile
from concourse.bass_utils import run_bass_kernel_spmd

F32 = mybir.dt.float32
F16 = mybir.dt.float16
U16 = mybir.dt.uint16

N_CORES = 8
B, S, D_EMBED = 2, 2048, 1024
N_HEADS = 16
D = 64  # d_head
NPAIR = (B * N_HEADS) // N_CORES  # 4 (b,h) pairs per core
NT = S // 128  # 16 sk-tiles
NCHUNK = S // 512  # 4 sq chunks
NG = 8  # exp groups per chunk, 2 sk-tiles each
DVE_GROUPS = (1, 3, 5, 7)  # these groups take the DVE fast-exp path
N_WARM = 4  # PE clock-gate warmup matmuls
# O-block schedule: (chunk_offset, sqb, g_slot) — at chunk t, slot g, emit
# the O block for chunk t-chunk_offset / sq-block sqb. The epilogue for a
# chunk is emitted right after its sqb=3 entry.
O_SCHED = ((1, 0, 2), (1, 1, 4), (1, 2, 6), (2, 3, 0))

# fp16 Schraudolph: bits(e^x) ~= round(2^10*log2(e)*x + 15*2^10 + sigma),
# sigma tuned to center the piecewise-linear sawtooth (RMS ~1.8%).
EXP_A = 1024.0 * float(np.log2(np.e))
EXP_B = 15.0 * 1024.0 - 59.5

_NC_CACHE = {}


def build_nc():
    nc = bacc.Bacc("TRN2", target_bir_lowering=False)
    # k and q ship interleaved in one tensor so each pair costs one DMA
    kq_d = nc.dram_tensor("kq", [NPAIR, 2, D, S], F16, kind="ExternalInput").ap()
    vs_d = nc.dram_tensor("vs", [NPAIR, S, D + 1], F16, kind="ExternalInput").ap()
    out_d = nc.dram_tensor("out", [NPAIR, S, D + 1], F16, kind="ExternalOutput").ap()

    with tile.TileContext(nc) as tc:
        with (
            tc.tile_pool(name="const", bufs=1) as const,
            tc.tile_pool(name="io", bufs=2) as io,
            tc.tile_pool(name="pt", bufs=3 * NG) as pt_pool,
            tc.tile_pool(name="ob", bufs=3) as ob_pool,
            tc.tile_pool(name="st_ps", bufs=3, space="PSUM") as st_ps,
            tc.tile_pool(name="o_ps", bufs=2, space="PSUM") as o_ps_pool,
        ):
            z_sb = const.tile([128, 512], F16)
            nc.vector.memset(z_sb, 0.0)
            # preload the ACT exp table (1283ns) under the initial DMA shadow
            warm_exp = const.tile([1, 2], F16)
            nc.scalar.activation(
                warm_exp, z_sb[0:1, 0:2], mybir.ActivationFunctionType.Exp
            )

            pairs = {}

            def alloc_pair(p):
                kq = io.tile([D, 2, S], F16, tag="kq", name=f"kq{p}")
                pairs[p] = {
                    "kq": kq,
                    "khT": kq[:, 0, :],
                    "qtT": kq[:, 1, :],
                    "v": io.tile([128, NT, D + 1], F16, tag="v", name=f"v{p}"),
                }

            def load_q(p):
                nc.sync.dma_start(
                    out=pairs[p]["kq"],
                    in_=kq_d[p].rearrange("t d s -> d t s"),
                )

            def load_v(p):
                nc.sync.dma_start(
                    out=pairs[p]["v"],
                    in_=vs_d[p].rearrange("(t r) d -> r t d", r=128),
                )

            # per-chunk pipeline state: t -> {pt tiles, o accum, pair tiles}
            cstate = {}

            def emit_scores_group(p, c, g, t):
                stt = st_ps.tile([128, 1024], F32, tag="st", name=f"st{t}_{g}")
                st_p = pairs[p]
                for j in range(2):
                    sk = 2 * g + j
                    nc.tensor.matmul(
                        stt[:, j * 512 : (j + 1) * 512],
                        st_p["khT"][:, sk * 128 : (sk + 1) * 128],
                        st_p["qtT"][:, c * 512 : (c + 1) * 512],
                        start=True,
                        stop=True,
                    )
                ptile = pt_pool.tile([128, 1024], F16, tag="pt", name=f"pt{t}_{g}")
                if g in DVE_GROUPS:
                    nc.vector.tensor_scalar(
                        out=ptile.bitcast(U16),
                        in0=stt,
                        scalar1=EXP_A,
                        scalar2=EXP_B,
                        op0=mybir.AluOpType.mult,
                        op1=mybir.AluOpType.add,
                    )
                else:
                    nc.scalar.activation(
                        ptile, stt, mybir.ActivationFunctionType.Exp
                    )
                cstate[t]["pt"].append(ptile)

            def emit_o_block(t, sqb, sk_lo=0, sk_hi=NT):
                cs = cstate[t]
                if "o" not in cs:
                    # first write below carries start=True, which zeroes the
                    # whole 2KB PSUM bank (verified on HW), so the other
                    # sq-blocks accumulate onto zeros with start=False
                    cs["o"] = o_ps_pool.tile(
                        [128, 4, D + 1], F32, tag="o", name=f"o{t}"
                    )
                    cs["zeroed"] = False
                o, v = cs["o"], cs["v"]
                for sk in range(sk_lo, sk_hi):
                    ptile = cs["pt"][sk // 2]
                    col = (sk % 2) * 512 + sqb * 128
                    nc.tensor.matmul(
                        o[:, sqb, :],
                        ptile[:, col : col + 128],
                        v[:, sk, :],
                        start=not cs["zeroed"],
                        stop=(sk == NT - 1),
                        skip_group_check=True,
                    )
                    cs["zeroed"] = True

            def emit_epilogue(t):
                cs = cstate.pop(t)
                ob = ob_pool.tile([128, 4, D + 1], F16, tag="ob", name=f"ob{t}")
                nc.scalar.mul(ob, cs["o"], 1.0 / 64.0)
                p, c = cs["p"], cs["c"]
                nc.sync.dma_start(
                    out=out_d[p].rearrange("(t r) d -> r t d", r=128)[
                        :, c * 4 : (c + 1) * 4, :
                    ],
                    in_=ob,
                )

            # ---- prologue: pair 0 loads + PE clock warmup ----
            # first quarter of khT/qtT ships first so chunk 0 can start while
            # the rest streams in behind it
            alloc_pair(0)
            nc.sync.dma_start(
                out=pairs[0]["kq"][:, :, 0:512],
                in_=kq_d[0].rearrange("t d s -> d t s")[:, :, 0:512],
            )
            nc.sync.dma_start(
                out=pairs[0]["kq"][:, :, 512:2048],
                in_=kq_d[0].rearrange("t d s -> d t s")[:, :, 512:2048],
            )
            load_v(0)
            for w in range(N_WARM):
                wt = st_ps.tile([128, 1024], F32, tag="st", name=f"warm{w}")
                nc.tensor.matmul(
                    wt[:, 0:512], z_sb[:, 0:128], z_sb, start=True, stop=True
                )

            # ---- main pipeline ----
            for p in range(NPAIR):
                for c in range(NCHUNK):
                    t = p * NCHUNK + c
                    cstate[t] = {"pt": [], "p": p, "c": c, "v": pairs[p]["v"]}
                    if p + 1 < NPAIR:
                        if c == 0:
                            alloc_pair(p + 1)
                            load_q(p + 1)
                        elif c == 1:
                            load_v(p + 1)
                    for g in range(NG):
                        emit_scores_group(p, c, g, t)
                        for off, sqb, gs in O_SCHED:
                            if gs == g and t - off >= 0:
                                emit_o_block(t - off, sqb)
                                if sqb == 3:
                                    emit_epilogue(t - off)
            # ---- tail: flush O blocks still owed by the schedule, newest
            # chunk in exp-trailing quarters ----
            t_last = NPAIR * NCHUNK - 1
            max_off = max(off for off, _, _ in O_SCHED)
            for t_o in range(t_last - max_off + 1, t_last):
                for off, sqb, gs in O_SCHED:
                    if t_o + off > t_last:
                        emit_o_block(t_o, sqb)
                        if sqb == 3:
                            emit_epilogue(t_o)
            for quarter in range(4):
                for sqb in range(4):
                    emit_o_block(
                        t_last, sqb, sk_lo=quarter * 4, sk_hi=(quarter + 1) * 4
                    )
            emit_epilogue(t_last)
    nc.finalize()
    return nc


def _host_prep(k, q, v, Wk, Wq, Wv):
    m1 = ((Wq.T @ Wk) / np.sqrt(np.float32(D))).astype(np.float32)

    def split_heads_T(x):
        return (
            x.reshape(B, S, N_HEADS, D)
            .transpose(0, 2, 3, 1)
            .reshape(B * N_HEADS, D, S)
        )

    khT = split_heads_T(k).astype(np.float16)
    qtT = (m1.T @ split_heads_T(q)).astype(np.float16)
    kq = np.ascontiguousarray(np.stack([khT, qtT], axis=1))  # [BH, 2, D, S]
    vh = v.reshape(B, S, N_HEADS, D).transpose(0, 2, 1, 3).reshape(B * N_HEADS, S, D)
    vt = np.empty((B * N_HEADS, S, D + 1), dtype=np.float16)
    vt[:, :, :D] = vh @ Wv.T
    vt[:, :, D] = 1.0
    return kq, vt


def kernel(k, q, v, Wk, Wq, Wv):
    k = np.asarray(k, dtype=np.float32)
    q = np.asarray(q, dtype=np.float32)
    v = np.asarray(v, dtype=np.float32)
    Wk = np.asarray(Wk, dtype=np.float32)
    Wq = np.asarray(Wq, dtype=np.float32)
    Wv = np.asarray(Wv, dtype=np.float32)

    kq, vt = _host_prep(k, q, v, Wk, Wq, Wv)

    if "nc" not in _NC_CACHE:
        _NC_CACHE["nc"] = build_nc()
    nc = _NC_CACHE["nc"]

    in_maps = []
    for core in range(N_CORES):
        sl = slice(core * NPAIR, (core + 1) * NPAIR)
        in_maps.append({"kq": kq[sl], "vs": vt[sl]})

    res = run_bass_kernel_spmd(nc, in_maps, core_ids=list(range(N_CORES)))
    outs = np.stack([r["out"] for r in res.results])  # [8, NPAIR, S, D+1]
    outs = outs.reshape(B * N_HEADS, S, D + 1).astype(np.float32)
    o = outs[:, :, :D] / outs[:, :, D:]
    out = (
        o.reshape(B, N_HEADS, S, D).transpose(0, 2, 1, 3).reshape(B, S, D_EMBED)
    )
    return out.astype(np.float32)
